# revision 16
# baseline (speedup 1.0000x reference)
"""3x3 NMS (maxpool + threshold + border) kernel for Trainium2, 8 NeuronCores.

Strategy:
  - Pure data parallel: 16 images -> 2 images per core on 8 cores.
  - Host zero-pads each image to H+2 rows so the kernel can load, per
    partition, R+2 consecutive rows (R=12 core rows + 1 halo row on each
    side) with a single overlapping strided DMA. Partition p of a tile
    holds padded rows p*R .. p*R+R+1 (= image rows p*R-1 .. p*R+R).
  - The image is split into NT column tiles (2-col halos) to fit SBUF.
  - Per tile, 5 vector-engine ops (all exact max/compare, no arithmetic):
      1. h1  = max(x<<1, x>>1)                 (horizontal neighbor max)
      2. hm  = max(h1, x)                      (horizontal 3-window max)
      3. v1  = max(hm_up, hm_dn)               (vertical neighbor max)
      4. M   = max(max(v1, 0.6), hm)           (fused scalar_tensor_tensor)
      5. mask= (x >= M) as u8                  (x==maxpool3x3(x) and x>=0.6)
  - Host: zero 10px border, np.nonzero -> (y, x) rows, exactly matching
    jnp.nonzero order (batch-major, then row, then col).
"""

import os
import sys

sys.path.insert(0, "/opt/trn_rl_repo")

import numpy as np

B, C, H, W = 16, 1, 1536, 1536
HP = H + 2                    # padded rows
N_CORES = 8
B_PER = B // N_CORES          # images per core
R = 12                        # rows per partition (128 * 12 = 1536)
NT = 4                        # column tiles per image
GPROWS = 6                    # rows of the v1 op computed on GPSIMD
V = W // NT                   # valid (output) columns per tile
PAD = 2                       # column halo on each side
REP_THR = 0.6

_CACHE = {}
LAST_RESULTS = None


def _build_program():
    import concourse.bass as bass
    import concourse.bacc as bacc
    import concourse.mybir as mybir
    from concourse.tile import TileContext

    f32 = mybir.dt.float32
    u8 = mybir.dt.uint8
    MAX = mybir.AluOpType.max
    GE = mybir.AluOpType.is_ge

    nc = bacc.Bacc()
    x_in = nc.declare_dram_parameter("x", [B_PER, HP, W], f32, isOutput=False)
    m_out = nc.declare_dram_parameter("mask", [B_PER, H, W], u8, isOutput=True)

    with TileContext(nc) as tc:
        with tc.tile_pool(name="pool", bufs=1) as pool:
            for img in range(B_PER):
                mi = m_out[img].rearrange("(p r) c -> p r c", r=R)
                for t in range(NT):
                    cs = max(t * V - PAD, 0)
                    ce = min(t * V + V + PAD, W)
                    WT = ce - cs
                    a = t * V - cs  # local col offset of the valid range

                    # overlapping strided view: partition p, row slot j,
                    # col c  ->  x[img, p*R + j, cs + c]
                    xi = bass.AP(x_in, img * HP * W + cs,
                                 [[R * W, 128], [W, R + 2], [1, WT]])

                    X = pool.tile([128, R + 2, WT], f32, tag="X", bufs=2,
                                  name=f"X_{img}_{t}")
                    A = pool.tile([128, R + 2, WT], f32, tag="A", bufs=2,
                                  name=f"A_{img}_{t}")
                    HM = pool.tile([128, R + 2, WT], f32, tag="HM", bufs=2,
                                   name=f"HM_{img}_{t}")
                    MSK = pool.tile([128, R, V], u8, tag="MSK", bufs=2,
                                    name=f"MSK_{img}_{t}")

                    nc.sync.dma_start(out=X[:, :, :], in_=xi)

                    # Vertical stage first: halo row slots are only ever READ
                    # (as shifted APs of X), so every compute op runs on
                    # exactly R row slices (saves the 2-halo-row compute tax).
                    # 1) vertical neighbor max
                    nc.vector.tensor_tensor(
                        A[:, 1:R + 1, :], X[:, 0:R, :], X[:, 2:R + 2, :], MAX)
                    # 2) vertical 3-window max vm
                    nc.vector.tensor_tensor(
                        HM[:, 1:R + 1, :], A[:, 1:R + 1, :],
                        X[:, 1:R + 1, :], MAX)
                    # 3) horizontal neighbor max of vm (valid cols 1..WT-2)
                    nc.vector.tensor_tensor(
                        A[:, 1:R + 1, 1:WT - 1], HM[:, 1:R + 1, 0:WT - 2],
                        HM[:, 1:R + 1, 2:WT], MAX)
                    # 4) M = max(max(h1, thr), vm)  (in-place into HM center)
                    nc.vector.scalar_tensor_tensor(
                        HM[:, 1:R + 1, 1:WT - 1], A[:, 1:R + 1, 1:WT - 1],
                        REP_THR, HM[:, 1:R + 1, 1:WT - 1], op0=MAX, op1=MAX)
                    # 5) mask = (x >= M); cols 0/1535 of the image read uninit
                    # M values on the first/last tile — border, zeroed on host
                    nc.vector.tensor_tensor(
                        MSK[:, :, :], X[:, 1:R + 1, a:a + V],
                        HM[:, 1:R + 1, a:a + V], GE)

                    nc.sync.dma_start(out=mi[:, :, t * V:(t + 1) * V],
                                      in_=MSK[:, :, :])
    nc.finalize()
    return nc


def _get_program():
    if "nc" not in _CACHE:
        _CACHE["nc"] = _build_program()
    return _CACHE["nc"]


def kernel(repeatability):
    global LAST_RESULTS
    from concourse.bass_utils import run_bass_kernel_spmd

    x = np.asarray(repeatability, dtype=np.float32).reshape(B, H, W)
    xp = np.zeros((B, HP, W), dtype=np.float32)
    xp[:, 1:H + 1, :] = x
    per_core = xp.reshape(N_CORES, B_PER, HP, W)
    in_maps = [{"x": np.ascontiguousarray(per_core[i])} for i in range(N_CORES)]

    nc = _get_program()
    res = run_bass_kernel_spmd(nc, in_maps, list(range(N_CORES)),
                               trace=bool(os.environ.get("NMS_TRACE")))
    LAST_RESULTS = res

    masks = np.stack([res.results[i]["mask"] for i in range(N_CORES)])
    mask_full = masks.reshape(B, C, H, W) != 0
    mask_full[:, :, :10, :] = False
    mask_full[:, :, -10:, :] = False
    mask_full[:, :, :, :10] = False
    mask_full[:, :, :, -10:] = False
    _, _, ys, xs = np.nonzero(mask_full)
    return np.stack([ys, xs]).astype(np.int32)


# revision 19
# speedup vs baseline: 1.3093x; 1.3093x over previous
"""3x3 NMS (maxpool + threshold + border) kernel for Trainium2, 8 NeuronCores.

Strategy:
  - Pure data parallel: 16 images -> 2 images per core on 8 cores.
  - Host zero-pads each image to H+2 rows so the kernel can load, per
    partition, R+2 consecutive rows (R=12 core rows + 1 halo row on each
    side) with a single overlapping strided DMA. Partition p of a tile
    holds padded rows p*R .. p*R+R+1 (= image rows p*R-1 .. p*R+R).
  - The image is split into NT column tiles (2-col halos) to fit SBUF.
  - Per tile, 5 vector-engine ops (all exact max/compare, no arithmetic):
      1. h1  = max(x<<1, x>>1)                 (horizontal neighbor max)
      2. hm  = max(h1, x)                      (horizontal 3-window max)
      3. v1  = max(hm_up, hm_dn)               (vertical neighbor max)
      4. M   = max(max(v1, 0.6), hm)           (fused scalar_tensor_tensor)
      5. mask= (x >= M) as u8                  (x==maxpool3x3(x) and x>=0.6)
  - Host: zero 10px border, np.nonzero -> (y, x) rows, exactly matching
    jnp.nonzero order (batch-major, then row, then col).
"""

import os
import sys

sys.path.insert(0, "/opt/trn_rl_repo")

import numpy as np

B, C, H, W = 16, 1, 1536, 1536
HP = H + 2                    # padded rows
N_CORES = 8
B_PER = B // N_CORES          # images per core
R = 12                        # rows per partition (128 * 12 = 1536)
NT = 4                        # column tiles per image
GPROWS = 6                    # rows of the v1 op computed on GPSIMD
V = W // NT                   # valid (output) columns per tile
PAD = 2                       # column halo on each side
REP_THR = 0.6

_CACHE = {}
LAST_RESULTS = None


def _build_program():
    import concourse.bass as bass
    import concourse.bacc as bacc
    import concourse.mybir as mybir
    from concourse.tile import TileContext

    f32 = mybir.dt.float32
    u8 = mybir.dt.uint8
    MAX = mybir.AluOpType.max
    GE = mybir.AluOpType.is_ge

    from slideops import make_ops
    M3, M3GE = make_ops()

    nc = bacc.Bacc()
    x_in = nc.declare_dram_parameter("x", [B_PER, HP, W], f32, isOutput=False)
    m_out = nc.declare_dram_parameter("mask", [B_PER, H, W], u8, isOutput=True)

    with TileContext(nc) as tc:
        with tc.tile_pool(name="pool", bufs=1) as pool:
            for img in range(B_PER):
                mi = m_out[img].rearrange("(p r) c -> p r c", r=R)
                for t in range(NT):
                    cs = max(t * V - PAD, 0)
                    ce = min(t * V + V + PAD, W)
                    WT = ce - cs
                    a = t * V - cs  # local col offset of the valid range

                    # overlapping strided view: partition p, row slot j,
                    # col c  ->  x[img, p*R + j, cs + c]
                    xi = bass.AP(x_in, img * HP * W + cs,
                                 [[R * W, 128], [W, R + 2], [1, WT]])

                    X = pool.tile([128, R + 2, WT], f32, tag="X", bufs=2,
                                  name=f"X_{img}_{t}")
                    VM = pool.tile([128, R + 2, WT], f32, tag="VM", bufs=2,
                                   name=f"VM_{img}_{t}")
                    MW = V if t == 0 else V + 2
                    MSK = pool.tile([128, R, V + 2], u8, tag="MSK", bufs=2,
                                    name=f"MSK_{img}_{t}")

                    nc.sync.dma_start(out=X[:, :, :], in_=xi)

                    # Pass 1: vertical sliding max3, streamed column-major so
                    # the window runs down rows. Stream pos j writes VM slot j
                    # = vmax centered on row slot j-1 (image row p*R + j - 2);
                    # slots 0,1 of each column are seam junk, never read.
                    nc.vector._custom_dve(
                        M3,
                        out=VM[:, :, :].transpose([0, 2, 1]),
                        in0=X[:, :, :].transpose([0, 2, 1]))

                    # Pass 2: horizontal sliding max3 over vm, fused with the
                    # 0.6 clamp and the (x >= M) compare, row-major streams.
                    # Junk at the first 2 cols of each row lands in discarded
                    # scratch cols (or border cols 0,1 for the first tile).
                    if t == 0:
                        # out col k = mask col k; window centered k
                        nc.vector._custom_dve(
                            M3GE,
                            out=MSK[:, :, 0:V],
                            in0=VM[:, 2:R + 2, 1:V + 1],
                            in1=X[:, 1:R + 1, 0:V],
                            s0=REP_THR)
                        nc.sync.dma_start(out=mi[:, :, 0:V],
                                          in_=MSK[:, :, 0:V])
                    else:
                        # out col k = mask col t*V-2+k; valid k in [2, V+2).
                        # On the last tile the final column's window would
                        # read past the image edge: shorten the stream by one
                        # and leave mask col W-1 (border, host-zeroed) junk.
                        SL = V + 2 if t < NT - 1 else V + 1
                        nc.vector._custom_dve(
                            M3GE,
                            out=MSK[:, :, 0:SL],
                            in0=VM[:, 2:R + 2, a - 1:a - 1 + SL],
                            in1=X[:, 1:R + 1, a - 2:a - 2 + SL],
                            s0=REP_THR)
                        nc.sync.dma_start(out=mi[:, :, t * V:(t + 1) * V],
                                          in_=MSK[:, :, 2:V + 2])
    nc.finalize()
    return nc


def _get_program():
    if "nc" not in _CACHE:
        _CACHE["nc"] = _build_program()
    return _CACHE["nc"]


def kernel(repeatability):
    global LAST_RESULTS
    from concourse.bass_utils import run_bass_kernel_spmd

    x = np.asarray(repeatability, dtype=np.float32).reshape(B, H, W)
    xp = np.zeros((B, HP, W), dtype=np.float32)
    xp[:, 1:H + 1, :] = x
    per_core = xp.reshape(N_CORES, B_PER, HP, W)
    in_maps = [{"x": np.ascontiguousarray(per_core[i])} for i in range(N_CORES)]

    nc = _get_program()
    res = run_bass_kernel_spmd(nc, in_maps, list(range(N_CORES)),
                               trace=bool(os.environ.get("NMS_TRACE")))
    LAST_RESULTS = res

    masks = np.stack([res.results[i]["mask"] for i in range(N_CORES)])
    mask_full = masks.reshape(B, C, H, W) != 0
    mask_full[:, :, :10, :] = False
    mask_full[:, :, -10:, :] = False
    mask_full[:, :, :, :10] = False
    mask_full[:, :, :, -10:] = False
    _, _, ys, xs = np.nonzero(mask_full)
    return np.stack([ys, xs]).astype(np.int32)


# revision 20
# speedup vs baseline: 1.5293x; 1.1681x over previous
"""3x3 NMS (maxpool + threshold + border) kernel for Trainium2, 8 NeuronCores.

Strategy:
  - Pure data parallel: 16 images -> 2 images per core on 8 cores.
  - Host zero-pads each image to H+2 rows so the kernel can load, per
    partition, R+2 consecutive rows (R=12 core rows + 1 halo row on each
    side) with a single overlapping strided DMA. Partition p of a tile
    holds padded rows p*R .. p*R+R+1 (= image rows p*R-1 .. p*R+R).
  - The image is split into NT column tiles (2-col halos) to fit SBUF.
  - Per tile, 5 vector-engine ops (all exact max/compare, no arithmetic):
      1. h1  = max(x<<1, x>>1)                 (horizontal neighbor max)
      2. hm  = max(h1, x)                      (horizontal 3-window max)
      3. v1  = max(hm_up, hm_dn)               (vertical neighbor max)
      4. M   = max(max(v1, 0.6), hm)           (fused scalar_tensor_tensor)
      5. mask= (x >= M) as u8                  (x==maxpool3x3(x) and x>=0.6)
  - Host: zero 10px border, np.nonzero -> (y, x) rows, exactly matching
    jnp.nonzero order (batch-major, then row, then col).
"""

import os
import sys

sys.path.insert(0, "/opt/trn_rl_repo")

import numpy as np

B, C, H, W = 16, 1, 1536, 1536
HP = H + 2                    # padded rows
N_CORES = 8
B_PER = B // N_CORES          # images per core
R = 12                        # rows per partition (128 * 12 = 1536)
NT = 4                        # column tiles per image
GPROWS = 6                    # rows of the v1 op computed on GPSIMD
V = W // NT                   # valid (output) columns per tile
PAD = 2                       # column halo on each side
REP_THR = 0.6

_CACHE = {}
LAST_RESULTS = None


def _build_program():
    import concourse.bass as bass
    import concourse.bacc as bacc
    import concourse.mybir as mybir
    from concourse.tile import TileContext

    f32 = mybir.dt.float32
    u8 = mybir.dt.uint8
    MAX = mybir.AluOpType.max
    GE = mybir.AluOpType.is_ge

    from slideops import make_ops
    M3, M3GE = make_ops()

    nc = bacc.Bacc()
    x_in = nc.declare_dram_parameter("x", [B_PER, HP, W], f32, isOutput=False)
    m_out = nc.declare_dram_parameter("mask", [B_PER, H, W], u8, isOutput=True)

    with TileContext(nc) as tc:
        with tc.tile_pool(name="pool", bufs=1) as pool:
            for img in range(B_PER):
                mi = m_out[img].rearrange("(p r) c -> p r c", r=R)
                for t in range(NT):
                    cs = max(t * V - PAD, 0)
                    ce = min(t * V + V + PAD, W)
                    WT = ce - cs
                    a = t * V - cs  # local col offset of the valid range

                    # overlapping strided view: partition p, row slot j,
                    # col c  ->  x[img, p*R + j, cs + c]
                    xi = bass.AP(x_in, img * HP * W + cs,
                                 [[R * W, 128], [W, R + 2], [1, WT]])

                    X = pool.tile([128, R + 2, WT], f32, tag="X", bufs=2,
                                  name=f"X_{img}_{t}")
                    VM = pool.tile([128, R + 2, WT], f32, tag="VM", bufs=2,
                                   name=f"VM_{img}_{t}")
                    MW = V if t == 0 else V + 2
                    MSK = pool.tile([128, R, V + 2], u8, tag="MSK", bufs=2,
                                    name=f"MSK_{img}_{t}")

                    nc.sync.dma_start(out=X[:, :, :], in_=xi)

                    # Pass 1: vertical 3-window max via two row-major TTs
                    # (a column-major sliding stream pays ~9 cycles per
                    # 14-element inner run — slower than two stock passes).
                    # VM slot j+2 = vmax centered image row p*R+j, matching
                    # what pass 2 expects.
                    nc.vector.tensor_tensor(
                        VM[:, 2:R + 2, :], X[:, 0:R, :], X[:, 2:R + 2, :], MAX)
                    nc.vector.tensor_tensor(
                        VM[:, 2:R + 2, :], VM[:, 2:R + 2, :],
                        X[:, 1:R + 1, :], MAX)

                    # Pass 2: horizontal sliding max3 over vm, fused with the
                    # 0.6 clamp and the (x >= M) compare, row-major streams.
                    # Junk at the first 2 cols of each row lands in discarded
                    # scratch cols (or border cols 0,1 for the first tile).
                    if t == 0:
                        # out col k = mask col k; window centered k
                        nc.vector._custom_dve(
                            M3GE,
                            out=MSK[:, :, 0:V],
                            in0=VM[:, 2:R + 2, 1:V + 1],
                            in1=X[:, 1:R + 1, 0:V],
                            s0=REP_THR)
                        nc.sync.dma_start(out=mi[:, :, 0:V],
                                          in_=MSK[:, :, 0:V])
                    else:
                        # out col k = mask col t*V-2+k; valid k in [2, V+2).
                        # On the last tile the final column's window would
                        # read past the image edge: shorten the stream by one
                        # and leave mask col W-1 (border, host-zeroed) junk.
                        SL = V + 2 if t < NT - 1 else V + 1
                        nc.vector._custom_dve(
                            M3GE,
                            out=MSK[:, :, 0:SL],
                            in0=VM[:, 2:R + 2, a - 1:a - 1 + SL],
                            in1=X[:, 1:R + 1, a - 2:a - 2 + SL],
                            s0=REP_THR)
                        nc.sync.dma_start(out=mi[:, :, t * V:(t + 1) * V],
                                          in_=MSK[:, :, 2:V + 2])
    nc.finalize()
    return nc


def _get_program():
    if "nc" not in _CACHE:
        _CACHE["nc"] = _build_program()
    return _CACHE["nc"]


def kernel(repeatability):
    global LAST_RESULTS
    from concourse.bass_utils import run_bass_kernel_spmd

    x = np.asarray(repeatability, dtype=np.float32).reshape(B, H, W)
    xp = np.zeros((B, HP, W), dtype=np.float32)
    xp[:, 1:H + 1, :] = x
    per_core = xp.reshape(N_CORES, B_PER, HP, W)
    in_maps = [{"x": np.ascontiguousarray(per_core[i])} for i in range(N_CORES)]

    nc = _get_program()
    res = run_bass_kernel_spmd(nc, in_maps, list(range(N_CORES)),
                               trace=bool(os.environ.get("NMS_TRACE")))
    LAST_RESULTS = res

    masks = np.stack([res.results[i]["mask"] for i in range(N_CORES)])
    mask_full = masks.reshape(B, C, H, W) != 0
    mask_full[:, :, :10, :] = False
    mask_full[:, :, -10:, :] = False
    mask_full[:, :, :, :10] = False
    mask_full[:, :, :, -10:] = False
    _, _, ys, xs = np.nonzero(mask_full)
    return np.stack([ys, xs]).astype(np.int32)


# revision 21
# speedup vs baseline: 1.5510x; 1.0142x over previous
"""3x3 NMS (maxpool + threshold + border) kernel for Trainium2, 8 NeuronCores.

Strategy:
  - Pure data parallel: 16 images -> 2 images per core on 8 cores.
  - Host zero-pads each image to H+2 rows so the kernel can load, per
    partition, R+2 consecutive rows (R=12 core rows + 1 halo row on each
    side) with a single overlapping strided DMA. Partition p of a tile
    holds padded rows p*R .. p*R+R+1 (= image rows p*R-1 .. p*R+R).
  - The image is split into NT column tiles (2-col halos) to fit SBUF.
  - Per tile, 5 vector-engine ops (all exact max/compare, no arithmetic):
      1. h1  = max(x<<1, x>>1)                 (horizontal neighbor max)
      2. hm  = max(h1, x)                      (horizontal 3-window max)
      3. v1  = max(hm_up, hm_dn)               (vertical neighbor max)
      4. M   = max(max(v1, 0.6), hm)           (fused scalar_tensor_tensor)
      5. mask= (x >= M) as u8                  (x==maxpool3x3(x) and x>=0.6)
  - Host: zero 10px border, np.nonzero -> (y, x) rows, exactly matching
    jnp.nonzero order (batch-major, then row, then col).
"""

import os
import sys

sys.path.insert(0, "/opt/trn_rl_repo")

import numpy as np

B, C, H, W = 16, 1, 1536, 1536
HP = H + 2                    # padded rows
N_CORES = 8
B_PER = B // N_CORES          # images per core
R = 12                        # rows per partition (128 * 12 = 1536)
NT = 4                        # column tiles per image
GPROWS = 6                    # rows of the v1 op computed on GPSIMD
V = W // NT                   # valid (output) columns per tile
PAD = 2                       # column halo on each side
REP_THR = 0.6

_CACHE = {}
LAST_RESULTS = None


def _build_program():
    import concourse.bass as bass
    import concourse.bacc as bacc
    import concourse.mybir as mybir
    from concourse.tile import TileContext

    f32 = mybir.dt.float32
    u8 = mybir.dt.uint8
    MAX = mybir.AluOpType.max
    GE = mybir.AluOpType.is_ge

    from slideops import make_ops
    M3, M3GE = make_ops()

    nc = bacc.Bacc()
    x_in = nc.declare_dram_parameter("x", [B_PER, HP, W], f32, isOutput=False)
    m_out = nc.declare_dram_parameter("mask", [B_PER, H, W], u8, isOutput=True)

    with TileContext(nc) as tc:
        with tc.tile_pool(name="pool", bufs=1) as pool:
            for img in range(B_PER):
                mi = m_out[img].rearrange("(p r) c -> p r c", r=R)
                for t in range(NT):
                    cs = max(t * V - PAD, 0)
                    ce = min(t * V + V + PAD, W)
                    WT = ce - cs
                    a = t * V - cs  # local col offset of the valid range

                    # overlapping strided view: partition p, row slot j,
                    # col c  ->  x[img, p*R + j, cs + c]
                    xi = bass.AP(x_in, img * HP * W + cs,
                                 [[R * W, 128], [W, R + 2], [1, WT]])

                    X = pool.tile([128, R + 2, WT], f32, tag="X", bufs=2,
                                  name=f"X_{img}_{t}")
                    VM = pool.tile([128, R + 2, WT], f32, tag="VM", bufs=1,
                                   name=f"VM_{img}_{t}")
                    MW = V if t == 0 else V + 2
                    MSK = pool.tile([128, R, V + 2], u8, tag="MSK", bufs=2,
                                    name=f"MSK_{img}_{t}")

                    nc.sync.dma_start(out=X[:, :, :], in_=xi)

                    # Pass 1: vertical 3-window max via two row-major TTs
                    # (a column-major sliding stream pays ~9 cycles per
                    # 14-element inner run — slower than two stock passes).
                    # VM slot j+2 = vmax centered image row p*R+j, matching
                    # what pass 2 expects.
                    nc.vector.tensor_tensor(
                        VM[:, 2:R + 2, :], X[:, 0:R, :], X[:, 2:R + 2, :], MAX)
                    nc.vector.tensor_tensor(
                        VM[:, 2:R + 2, :], VM[:, 2:R + 2, :],
                        X[:, 1:R + 1, :], MAX)

                    # Pass 2: horizontal sliding max3 over vm, fused with the
                    # 0.6 clamp and the (x >= M) compare, row-major streams.
                    # Junk at the first 2 cols of each row lands in discarded
                    # scratch cols (or border cols 0,1 for the first tile).
                    if t == 0:
                        # out col k = mask col k; window centered k
                        nc.vector._custom_dve(
                            M3GE,
                            out=MSK[:, :, 0:V],
                            in0=VM[:, 2:R + 2, 1:V + 1],
                            in1=X[:, 1:R + 1, 0:V],
                            s0=REP_THR)
                        nc.sync.dma_start(out=mi[:, :, 0:V],
                                          in_=MSK[:, :, 0:V])
                    else:
                        # out col k = mask col t*V-2+k; valid k in [2, V+2).
                        # On the last tile the final column's window would
                        # read past the image edge: shorten the stream by one
                        # and leave mask col W-1 (border, host-zeroed) junk.
                        SL = V + 2 if t < NT - 1 else V + 1
                        nc.vector._custom_dve(
                            M3GE,
                            out=MSK[:, :, 0:SL],
                            in0=VM[:, 2:R + 2, a - 1:a - 1 + SL],
                            in1=X[:, 1:R + 1, a - 2:a - 2 + SL],
                            s0=REP_THR)
                        nc.sync.dma_start(out=mi[:, :, t * V:(t + 1) * V],
                                          in_=MSK[:, :, 2:V + 2])
    nc.finalize()
    return nc


def _get_program():
    if "nc" not in _CACHE:
        _CACHE["nc"] = _build_program()
    return _CACHE["nc"]


def kernel(repeatability):
    global LAST_RESULTS
    from concourse.bass_utils import run_bass_kernel_spmd

    x = np.asarray(repeatability, dtype=np.float32).reshape(B, H, W)
    xp = np.zeros((B, HP, W), dtype=np.float32)
    xp[:, 1:H + 1, :] = x
    per_core = xp.reshape(N_CORES, B_PER, HP, W)
    in_maps = [{"x": np.ascontiguousarray(per_core[i])} for i in range(N_CORES)]

    nc = _get_program()
    res = run_bass_kernel_spmd(nc, in_maps, list(range(N_CORES)),
                               trace=bool(os.environ.get("NMS_TRACE")))
    LAST_RESULTS = res

    masks = np.stack([res.results[i]["mask"] for i in range(N_CORES)])
    mask_full = masks.reshape(B, C, H, W) != 0
    mask_full[:, :, :10, :] = False
    mask_full[:, :, -10:, :] = False
    mask_full[:, :, :, :10] = False
    mask_full[:, :, :, -10:] = False
    _, _, ys, xs = np.nonzero(mask_full)
    return np.stack([ys, xs]).astype(np.int32)
